# revision 2
# baseline (speedup 1.0000x reference)
"""Causal multi-head attention on 8 trn2 NeuronCores — bf16 redesign.

Sharding: core = (batch b in {0,1}) x (head-group g in {0..3}; 4 heads each).
QKV weights column-sharded, Wo row-sharded (Megatron TP); each core emits a
bf16 partial output for its batch; the host sums the 4 partials per batch in
fp32 and adds the output bias.

All matmul operands are bf16 (PSUM accumulation stays fp32) which runs the PE
at 1 cycle/row even for narrow moving dims.  Scores are computed transposed
(k,q) flash-style; probabilities pT stay resident in SBUF for a whole q-block.
AV is computed q-major (out[q, d] with pT as the stationary operand) at full
128-partition PE utilization; v is padded with a ones column so each head's
softmax denominator lands in the av tile for free.  Normalization is then a
per-partition reciprocal + broadcast multiply on DVE (no PE broadcast matmul).
The normalized q-major attention tile is transposed back to feature-major via
the DMA XBAR (128x128 bf16 blocks), feeding the output projection.  Causality
uses partial-width matmuls plus a 128x128 triangle mask multiply on diagonal
chunks (on the idle Pool engine).
"""

import numpy as np

B, S, E, H, D = 2, 2048, 1024, 16, 64
NCORES = 8
G = 4            # head-groups (cores per batch)
HPG = H // G     # heads per core = 4
FS = HPG * D     # feature slice per core = 256
P = 128
QB = 512         # query block (matmul moving width)
NQB = S // QB    # 4
NKC = S // P     # 16 k-chunks
EC = E // P      # 8 contraction chunks for projections

_cache = {}


def _split_waits(nc, mybir, max_waits=1):
    """This walrus build encodes at most one sem-wait per instruction.
    Hoist extra waits onto NOPs inserted before the instruction in the same
    engine stream (same basic block => order preserved)."""
    uid = [0]
    for fn in nc.m.functions:
        for bb in fn.blocks:
            new = []
            changed = False
            for inst in bb.instructions:
                si = inst.sync_info
                if si is not None and len(si.on_wait) > max_waits:
                    waits = list(si.on_wait)
                    head, tail = waits[:-max_waits], waits[-max_waits:]
                    for k in range(0, len(head), max_waits):
                        nop = mybir.InstNoOp(name=f"WSPLIT-{uid[0]}", ins=[], outs=[])
                        uid[0] += 1
                        nop.engine = inst.engine
                        nop.sync_info = mybir.SyncInfo(
                            on_wait=head[k:k + max_waits], on_update=[])
                        new.append(nop)
                    inst.sync_info = mybir.SyncInfo(
                        on_wait=tail, on_update=list(si.on_update))
                    changed = True
                new.append(inst)
            if changed:
                bb.instructions = new


def _build(reps=1):
    key = ("nc", reps)
    if key in _cache:
        return _cache[key]
    import os
    no_warm = bool(os.environ.get("ABL_NOWARM"))
    mask_dve = bool(os.environ.get("V2_MASK_DVE"))

    import concourse.bass as bass
    import concourse.mybir as mybir
    import concourse.tile as tile

    F32 = mybir.dt.float32
    BF16 = mybir.dt.bfloat16
    EXP = mybir.ActivationFunctionType.Exp

    nc = bass.Bass("TRN2", target_bir_lowering=False, debug=False)

    xt_d = nc.dram_tensor("xt", [E, S], BF16, kind="ExternalInput")
    wq_d = nc.dram_tensor("wqt", [P, EC, FS], BF16, kind="ExternalInput")
    wk_d = nc.dram_tensor("wkt", [P, EC, FS], BF16, kind="ExternalInput")
    wv_d = nc.dram_tensor("wvt", [P, EC, FS], BF16, kind="ExternalInput")
    wo_d = nc.dram_tensor("wot", [P, FS // P, E], BF16, kind="ExternalInput")
    bq_d = nc.dram_tensor("bq", [P, 2], F32, kind="ExternalInput")
    bk_d = nc.dram_tensor("bk", [P, 2], F32, kind="ExternalInput")
    bv_d = nc.dram_tensor("bvb", [P, FS], F32, kind="ExternalInput")   # pre-broadcast
    mask_d = nc.dram_tensor("mask", [P, P], BF16, kind="ExternalInput")  # tri: 1 if j>=k
    out_d = nc.dram_tensor("outt", [E, S], BF16, kind="ExternalOutput")

    with tile.TileContext(nc) as tc, \
         nc.allow_low_precision(reason="bf16 operand rounding is intended"):
      for _rep in range(reps):
        R = f"r{_rep}"
        with tc.tile_pool(name=f"big{R}", bufs=1) as big, \
             tc.tile_pool(name=f"small{R}", bufs=1) as small:

            # ---- resident inputs ----
            wq_t = big.tile([P, EC, FS], BF16, tag="wq")
            nc.scalar.dma_start(wq_t[:], wq_d[:])
            wk_t = big.tile([P, EC, FS], BF16, tag="wk")
            nc.sync.dma_start(wk_t[:], wk_d[:])
            wv_t = big.tile([P, EC, FS], BF16, tag="wv")
            nc.scalar.dma_start(wv_t[:], wv_d[:])
            wo_t = big.tile([P, 2, E], BF16, tag="wo")
            nc.sync.dma_start(wo_t[:], wo_d[:])
            xt = [big.tile([P, S], BF16, name=f"xt{c}{R}", tag=f"xt{c}")
                  for c in range(EC)]
            HB = S // 2
            for sc in range(2):
                for c in range(EC):
                    eng = nc.scalar if c % 2 == 0 else nc.sync
                    eng.dma_start(
                        xt[c][:, sc * HB:sc * HB + HB],
                        xt_d[bass.ts(c, P), sc * HB:sc * HB + HB])
            bq_t = small.tile([P, 2], F32, tag="bq")
            nc.sync.dma_start(bq_t[:], bq_d[:])
            bk_t = small.tile([P, 2], F32, tag="bk")
            nc.sync.dma_start(bk_t[:], bk_d[:])
            bv_t = small.tile([P, FS], F32, tag="bv")
            nc.sync.dma_start(bv_t[:], bv_d[:])
            mask_t = small.tile([P, P], BF16, tag="mask")
            nc.sync.dma_start(mask_t[:], mask_d[:])
            warm_f = small.tile([P, QB], F32, tag="warmf")
            nc.any.memset(warm_f[:], 0.5)
            warm_z = small.tile([P, QB], BF16, tag="warmz")
            nc.vector.tensor_copy(warm_z[:], warm_f[:])

            # ---- outputs of phase 1 (resident) ----
            qT = [big.tile([P, S], BF16, name=f"qT{f}{R}", tag=f"qT{f}") for f in range(2)]
            kT = [big.tile([P, S], BF16, name=f"kT{f}{R}", tag=f"kT{f}") for f in range(2)]
            vpad = [big.tile([P, HPG, D + 1], BF16, name=f"vp{c}{R}", tag=f"vp{c}")
                    for c in range(NKC)]
            # attention output, feature-major: partition p, slot j holds
            # feature f = 128*j + p (the XBAR transpose writes transposed
            # rows in 128-row blocks across the extra output dim).
            attnT = big.tile([P, 2, S], BF16, name=f"aT{R}", tag="aT")

            # ones columns for the AV denominator trick (Pool engine, idle)
            for sv in range(NKC):
                nc.gpsimd.memset(vpad[sv][:, :, D:D + 1], 1.0)

            # ---- fused pipeline: projections interleaved with attention ----
            # PE emission order is execution order, so projection matmul
            # groups for sequence-block sc=qb+1 are interleaved into q-block
            # qb's score/exp chunk loop at c granularity: while ACT churns
            # through the exps of q-block qb, PE fills with next-block
            # projections instead of stalling on the score->exp->AV chain.
            with tc.tile_pool(name=f"pbig{R}", bufs=2, space="PSUM") as pbig, \
                 tc.tile_pool(name=f"psc{R}", bufs=2, space="PSUM") as psc, \
                 tc.tile_pool(name=f"pav{R}", bufs=2, space="PSUM") as pav, \
                 tc.tile_pool(name=f"pt{R}", bufs=NKC + 3) as ptp, \
                 tc.tile_pool(name=f"rc{R}", bufs=4) as rcp, \
                 tc.tile_pool(name=f"aq{R}", bufs=4) as aqp, \
                 tc.tile_pool(name=f"ot{R}", bufs=4) as otp:

                def proj_groups(sc):
                    """Yield thunks, each emitting one PSUM-group of the
                    projections for sequence block sc (4 QK + 4 V groups)."""
                    def qk(fc, dst, w, bias):
                        def go():
                            ps = pbig.tile([P, QB], F32, tag="pbig")
                            for ec in range(EC):
                                nc.tensor.matmul(
                                    ps[:], w[:, ec, bass.ts(fc, P)],
                                    xt[ec][:, bass.ts(sc, QB)],
                                    start=(ec == 0), stop=(ec == EC - 1))
                            nc.vector.tensor_add(
                                dst[fc][:, bass.ts(sc, QB)], ps[:],
                                bias[:, fc:fc + 1].to_broadcast((P, QB)))
                        return go

                    def v(sv):
                        def go():
                            ps = pbig.tile([P, QB], F32, tag="pbig")
                            for ec in range(EC):
                                nc.tensor.matmul(
                                    ps[:, 0:FS], xt[ec][:, bass.ts(sv, P)],
                                    wv_t[:, ec, :],
                                    start=(ec == 0), stop=(ec == EC - 1))
                            psv = ps[:, 0:FS].rearrange("p (h d) -> p h d", h=HPG)
                            bvv = bv_t.rearrange("p (h d) -> p h d", h=HPG)
                            nc.vector.tensor_add(vpad[sv][:, :, 0:D], psv[:], bvv[:])
                        return go

                    for fc in range(2):
                        for dst, w, bias in ((qT, wq_t, bq_t), (kT, wk_t, bk_t)):
                            yield qk(fc, dst, w, bias)
                    for sv in range(4 * sc, 4 * sc + 4):
                        yield v(sv)

                def emit_outproj(qb):
                    for mp in range(EC // 2):
                        ot = otp.tile([P, 2, QB], BF16, tag="ot")
                        for mh in range(2):
                            m = 2 * mp + mh
                            po = pbig.tile([P, QB], F32, tag="pbig")
                            nc.tensor.matmul(po[:], wo_t[:, 0, bass.ts(m, P)],
                                             attnT[:, 0, bass.ts(qb, QB)],
                                             start=True, stop=False)
                            nc.tensor.matmul(po[:], wo_t[:, 1, bass.ts(m, P)],
                                             attnT[:, 1, bass.ts(qb, QB)],
                                             start=False, stop=True)
                            nc.vector.tensor_copy(ot[:, mh, :], po[:])
                        # batched store of two 128-row output chunks (SWDGE
                        # ring on Pool keeps the HWDGE queues free for
                        # transposes and input streams)
                        nc.gpsimd.dma_start(
                            out_d[2 * mp * P:2 * mp * P + 2 * P,
                                  bass.ts(qb, QB)].rearrange(
                                      "(j p) q -> p j q", j=2, p=P),
                            ot[:])

                # PE warmup: matmuls with no DMA dependency open the HAM
                # clock gate (1.2->2.4GHz) while the input DMAs stream in.
                if not no_warm:
                    wps = pbig.tile([P, QB], F32, tag="pbig", name=f"wps{R}")
                    for wi in range(16):
                        nc.tensor.matmul(wps[:], warm_z[:, 0:P], warm_z[:],
                                         start=(wi == 0), stop=(wi == 15))

                # phase-0 projections for the first q-block
                for go in proj_groups(0):
                    go()

                for qb in range(NQB):
                    q0 = qb * QB
                    nch = (q0 + QB) // P
                    pending = list(proj_groups(qb + 1)) if qb + 1 < NQB else []
                    # scores + exp for every k-chunk, head-PAIRED so each ACT
                    # instruction covers 1024 columns (halves fixed overheads)
                    pts = []
                    for c in range(nch):
                        delta = max(0, c * P - q0)
                        pT4 = ptp.tile([P, HPG, QB], BF16, tag="pT",
                                       name=f"pT{qb}_{c}{R}")
                        for fc in range(2):
                            sp = psc.tile([P, 2, QB], F32, tag="sc")
                            for hh in range(2):
                                ro = hh * D
                                nc.tensor.matmul(
                                    sp[:, hh, delta:QB],
                                    kT[fc][ro:ro + D, bass.ts(c, P)],
                                    qT[fc][ro:ro + D, q0 + delta:q0 + QB],
                                    start=True, stop=True)
                            nc.scalar.activation(
                                pT4[:, 2 * fc:2 * fc + 2, delta:QB],
                                sp[:, :, delta:QB], EXP, scale=0.125)
                            if c * P >= q0:
                                meng = nc.vector if mask_dve else nc.gpsimd
                                meng.tensor_mul(
                                    pT4[:, 2 * fc:2 * fc + 2, delta:delta + P],
                                    pT4[:, 2 * fc:2 * fc + 2, delta:delta + P],
                                    mask_t[:, None, :].to_broadcast((P, 2, P)))
                        pts.append(pT4)
                        # interleave next-block projection groups to keep PE fed
                        ngrp = (len(pending) + nch - 1 - c) // (nch - c) \
                            if pending else 0
                        for _ in range(ngrp):
                            pending.pop(0)()
                    for go in pending:
                        go()
                    if qb > 0:
                        emit_outproj(qb - 1)
                    # q-major AV per 128-row q-chunk, all 4 heads
                    for j in range(4):
                        gq = 4 * qb + j
                        off = j * P
                        av = pav.tile([P, HPG, D + 1], F32, tag="av")
                        for h in range(HPG):
                            for c in range(gq + 1):
                                nc.tensor.matmul(
                                    av[:, h, :],
                                    pts[c][:, h, off:off + P],
                                    vpad[c][:, h, :],
                                    start=(c == 0), stop=(c == gq))
                        rc = rcp.tile([P, HPG, 1], F32, tag="rcp")
                        nc.vector.reciprocal(rc[:], av[:, :, D:D + 1])
                        aq = aqp.tile([P, HPG, D], BF16, tag="aq")
                        nc.vector.tensor_mul(
                            aq[:], av[:, :, 0:D],
                            rc[:].to_broadcast((P, HPG, D)))
                        # transpose q-major attn back to feature-major via
                        # the DMA XBAR: [128, 256] -> [128, 2, 128] (feature
                        # pairs interleaved across partition slots)
                        nc.sync.dma_start(
                            attnT[:, :, gq * P:(gq + 1) * P],
                            aq[:], transpose=True)
                emit_outproj(NQB - 1)

    if not os.environ.get("V2_NO_WSPLIT"):
        _split_waits(nc, mybir)
    _cache[key] = nc
    return nc


def _ilv(w):
    """(C*128, N) -> (128, C, N): partition-major interleave for plain DMA."""
    c = w.shape[0] // P
    return np.ascontiguousarray(w.reshape(c, P, w.shape[1]).transpose(1, 0, 2))


def _in_maps(x, Wq, bq, Wk, bk, Wv, bv, Wo, bo):
    import ml_dtypes
    f32 = np.float32
    bf16 = ml_dtypes.bfloat16
    xT = [np.ascontiguousarray(x[b].T).astype(bf16) for b in range(B)]
    WqT = np.ascontiguousarray(Wq.T, dtype=f32)
    WkT = np.ascontiguousarray(Wk.T, dtype=f32)
    WvT = np.ascontiguousarray(Wv.T, dtype=f32)
    tri = np.triu(np.ones((P, P), dtype=f32))  # [k, j] = 1 if j >= k
    maps = []
    for core in range(NCORES):
        b, g = divmod(core, G)
        fs = slice(g * FS, (g + 1) * FS)
        maps.append({
            "xt": xT[b],
            "wqt": _ilv(WqT[:, fs]).astype(bf16),
            "wkt": _ilv(WkT[:, fs]).astype(bf16),
            "wvt": _ilv(WvT[:, fs]).astype(bf16),
            "wot": _ilv(Wo[:, fs].T).astype(bf16),
            "bq": np.ascontiguousarray(bq[fs].reshape(2, P).T),
            "bk": np.ascontiguousarray(bk[fs].reshape(2, P).T),
            "bvb": np.broadcast_to(bv[fs], (P, FS)).copy(),
            "mask": tri.astype(bf16),
        })
    return maps


def _runner(reps=1):
    """Compile once; return (exec_fn, put_fn, time_fn)."""
    rkey = ("run", reps)
    if rkey in _cache:
        return _cache[rkey]

    import jax
    from jax.experimental.shard_map import shard_map
    from jax.sharding import Mesh, NamedSharding, PartitionSpec

    import concourse.mybir as mybir
    from concourse.bass2jax import (
        _bass_exec_p,
        install_neuronx_cc_hook,
        partition_id_tensor,
    )

    nc = _build(reps)
    install_neuronx_cc_hook()

    partition_name = nc.partition_id_tensor.name if nc.partition_id_tensor else None
    in_names, out_names, out_avals, zero_outs = [], [], [], []
    for alloc in nc.m.functions[0].allocations:
        if not isinstance(alloc, mybir.MemoryLocationSet):
            continue
        name = alloc.memorylocations[0].name
        if alloc.kind == "ExternalInput":
            if name != partition_name:
                in_names.append(name)
        elif alloc.kind == "ExternalOutput":
            shape = tuple(alloc.tensor_shape)
            dtype = mybir.dt.np(alloc.dtype)
            out_names.append(name)
            out_avals.append(jax.core.ShapedArray(shape, dtype))
            zero_outs.append(np.zeros(shape, dtype))
    n_params = len(in_names)
    all_in_names = list(in_names) + list(out_names)
    if partition_name is not None:
        all_in_names.append(partition_name)

    def _body(*args):
        operands = list(args)
        if partition_name is not None:
            operands.append(partition_id_tensor())
        outs = _bass_exec_p.bind(
            *operands,
            out_avals=tuple(out_avals),
            in_names=tuple(all_in_names),
            out_names=tuple(out_names),
            lowering_input_output_aliases=(),
            sim_require_finite=True,
            sim_require_nnan=True,
            nc=nc,
        )
        return tuple(outs)

    devices = jax.devices()[:NCORES]
    mesh = Mesh(np.asarray(devices), ("core",))
    n_ops = n_params + len(out_names)
    sharded = jax.jit(
        shard_map(
            _body, mesh=mesh,
            in_specs=(PartitionSpec("core"),) * n_ops,
            out_specs=(PartitionSpec("core"),) * len(out_names),
            check_rep=False,
        ),
        keep_unused=True,
    )
    shard = NamedSharding(mesh, PartitionSpec("core"))

    def put_fn(maps):
        concat = [
            np.concatenate([np.asarray(maps[c][n]) for c in range(NCORES)], axis=0)
            for n in in_names
        ] + [
            np.concatenate([z] * NCORES, axis=0) for z in zero_outs
        ]
        return [jax.device_put(a, shard) for a in concat]

    def exec_fn(args):
        out_arrs = sharded(*args)
        jax.block_until_ready(out_arrs)
        return [
            {
                n: np.asarray(out_arrs[i]).reshape(NCORES, *out_avals[i].shape)[c]
                for i, n in enumerate(out_names)
            }
            for c in range(NCORES)
        ]

    def time_fn(args):
        out_arrs = sharded(*args)
        jax.block_until_ready(out_arrs)

    _cache[rkey] = (exec_fn, put_fn, time_fn)
    return _cache[rkey]


def _assemble(results, bo):
    out = np.empty((B, S, E), dtype=np.float32)
    for b in range(B):
        acc = results[b * G]["outt"].astype(np.float32)
        for g in range(1, G):
            acc = acc + results[b * G + g]["outt"].astype(np.float32)
        out[b] = acc.T + bo
    return out


def kernel(x, Wq, bq, Wk, bk, Wv, bv, Wo, bo):
    exec_fn, put_fn, _ = _runner()
    maps = _in_maps(x, Wq, bq, Wk, bk, Wv, bv, Wo, bo)
    args = put_fn(maps)
    if not _cache.get("warm"):
        exec_fn(args)
        _cache["warm"] = True
    results = exec_fn(args)
    return _assemble(results, bo)
